# revision 5
# baseline (speedup 1.0000x reference)
"""Trainium2 Bass kernel for nn_DifferentiableFeatureAligner.

Full cross-attention aligner: encode 6-feature descriptors of ego/other
(Linear(6,128)+ReLU+LN+Linear(128,128)), softmax(Q K^T / 0.1) attention
over 16384 keys, weighted sums of other_cls/other_reg channels.

Sharding: sequence-parallel over the 16384 ego queries -> 2048 per core
on 8 NeuronCores; other-side tensors replicated. Host only slices and
concatenates.

Score precision: plain fp32 matmul is 4 cycles/column on the PE. Instead
descriptors are split hi/lo into two bf16 tensors and scores computed as
qh*kh + ql*kh + qh*kl (3 bf16 passes, 1 cycle/column each, ~16-bit
mantissa => ~8e-4 end-to-end error). The softmax row max is found with a
single cheap bf16 pass (softmax is shift-invariant, the max needs no
precision). exp goes through float32r for the 1-cycle AV matmul.
"""
import numpy as np
from contextlib import ExitStack

import concourse.bacc as bacc
import concourse.tile as tile
import concourse.mybir as mybir
import concourse.bass_isa as bass_isa
from concourse.bass_utils import run_bass_kernel_spmd
from concourse.masks import make_identity

F32 = mybir.dt.float32
F32R = mybir.dt.float32r
BF16 = mybir.dt.bfloat16
AF = mybir.ActivationFunctionType
ALU = mybir.AluOpType
PI = float(np.pi)

N_CORES = 8
HW = 16384
NQ = HW // N_CORES      # queries per core
E = 128
TEMP_INV = 10.0
LN_EPS = 1e-5

_cache: dict = {}


def _encode_side(nc, P, n, feat_dram, scratch_dram, descH, descL):
    """Encode one side: feat_dram (6,n) -> descT [128,n] split into
    bf16 hi (descH) and bf16 lo residual (descL).  n multiple of 512."""
    nf = n // 128
    sb1, sbBig, hrp = P["sb1"], P["sbBig"], P["hrp"]
    psA, psT, psD = P["psA"], P["psT"], P["psD"]

    feat = feat_dram.ap()
    feat_pf = feat.rearrange("c (p f) -> c p f", p=128)

    # ---- feature rows ----
    xT = sbBig.tile([7, n], F32, tag=f"xT{n}")
    nc.sync.dma_start(xT[0:3, :], feat[0:3, :])

    th = sb1.tile([128, nf], F32, tag="th")
    nc.sync.dma_start(th[:], feat_pf[3])
    w0 = sb1.tile([128, nf], F32, tag="w0")
    nc.vector.add_range_wrap(w0[:], th[:], 0.0, PI, 2 * PI)
    sn = sb1.tile([128, nf], F32, tag="sn")
    nc.scalar.activation(sn[:], w0[:], AF.Sin)
    w1t = sb1.tile([128, nf], F32, tag="w1t")
    nc.vector.add_range_wrap(w1t[:], th[:], PI / 2, PI, 2 * PI)
    cs = sb1.tile([128, nf], F32, tag="cs")
    nc.scalar.activation(cs[:], w1t[:], AF.Sin)
    c0 = sb1.tile([128, nf], F32, tag="c0")
    nc.sync.dma_start(c0[:], feat_pf[4])
    c1 = sb1.tile([128, nf], F32, tag="c1")
    nc.sync.dma_start(c1[:], feat_pf[5])
    mx = sb1.tile([128, nf], F32, tag="mx")
    nc.vector.tensor_max(mx[:], c0[:], c1[:])

    # bounce through DRAM scratch to turn [128,nf] tiles into single rows
    ones_t = sb1.tile([128, nf], F32, tag="ones_t")
    nc.vector.memset(ones_t[:], 1.0)
    sc = scratch_dram.ap()
    nc.sync.dma_start(sc[0:1, :], sn[:])
    nc.sync.dma_start(sc[1:2, :], cs[:])
    nc.sync.dma_start(sc[2:3, :], mx[:])
    nc.sync.dma_start(sc[3:4, :], ones_t[:])
    nc.sync.dma_start(xT[3:7, :], sc[:])

    # ---- encode: per 512-col group of 4 windows ----
    for g in range(n // 512):
        st1 = sb1.tile([128, 4], F32, tag="st1")
        st2 = sb1.tile([128, 4], F32, tag="st2")
        hrs = []
        for w in range(4):
            win = g * 4 + w
            ph = psA.tile([128, 128], F32, tag="ph")
            nc.tensor.matmul(ph[:], xT[:, win * 128:(win + 1) * 128],
                             P["w17"][:], start=True, stop=True)
            hr = hrp.tile([128, 128], F32, tag="hr")
            nc.scalar.activation(hr[:], ph[:], AF.Relu,
                                 accum_out=st1[:, w:w + 1])
            sq = sb1.tile([128, 128], F32, tag="sq")
            nc.scalar.activation(sq[:], hr[:], AF.Square,
                                 accum_out=st2[:, w:w + 1])
            hrs.append(hr)
        mu4 = sb1.tile([128, 4], F32, tag="mu4")
        nc.vector.tensor_scalar_mul(mu4[:], st1[:], 1.0 / E)
        ex2 = sb1.tile([128, 4], F32, tag="ex2")
        nc.vector.tensor_scalar_mul(ex2[:], st2[:], 1.0 / E)
        mu2 = sb1.tile([128, 4], F32, tag="mu2")
        nc.vector.tensor_mul(mu2[:], mu4[:], mu4[:])
        var4 = sb1.tile([128, 4], F32, tag="var4")
        nc.vector.tensor_sub(var4[:], ex2[:], mu2[:])
        sd4 = sb1.tile([128, 4], F32, tag="sd4")
        nc.scalar.activation(sd4[:], var4[:], AF.Sqrt, bias=P["epsc"][:])
        rstd4 = sb1.tile([128, 4], F32, tag="rstd4")
        nc.vector.reciprocal(rstd4[:], sd4[:])

        hT = sb1.tile([128, 512], F32, tag="hT")
        for w in range(4):
            hn = sb1.tile([128, 128], F32, tag="hn")
            nc.vector.tensor_scalar(hn[:], hrs[w][:], mu4[:, w:w + 1],
                                    rstd4[:, w:w + 1],
                                    op0=ALU.subtract, op1=ALU.mult)
            pt = psT.tile([128, 128], F32, tag="pt")
            nc.tensor.transpose(pt[:], hn[:], P["ident"][:])
            nc.vector.tensor_scalar(hT[:, w * 128:(w + 1) * 128], pt[:],
                                    P["gcol"][:], P["bcol"][:],
                                    op0=ALU.mult, op1=ALU.add)
        pd = psD.tile([128, 512], F32, tag="pd")
        nc.tensor.matmul(pd[:], P["w2t"][:], hT[:], start=True, stop=True)
        dc = sb1.tile([128, 512], F32, tag="dc")
        nc.scalar.activation(dc[:], pd[:], AF.Identity, bias=P["b2col"][:])
        gs = slice(g * 512, (g + 1) * 512)
        nc.vector.tensor_copy(descH[:, gs], dc[:])
        nc.vector.tensor_sub(descL[:, gs], dc[:], descH[:, gs])


def build():
    if "nc" in _cache:
        return _cache["nc"]
    nc = bacc.Bacc("TRN2", target_bir_lowering=False, debug=False)

    QFEAT = nc.dram_tensor("qfeat", [6, NQ], F32, kind="ExternalInput")
    KFEAT = nc.dram_tensor("kfeat", [6, HW], F32, kind="ExternalInput")
    KV = nc.dram_tensor("kv", [17, HW], F32, kind="ExternalInput")
    W17 = nc.dram_tensor("w17", [7, E], F32, kind="ExternalInput")
    W2 = nc.dram_tensor("w2", [E, E], F32, kind="ExternalInput")
    LNG = nc.dram_tensor("lng", [E, 1], F32, kind="ExternalInput")
    LNB = nc.dram_tensor("lnb", [E, 1], F32, kind="ExternalInput")
    B2 = nc.dram_tensor("b2", [E, 1], F32, kind="ExternalInput")
    ALN = nc.dram_tensor("aln", [16, NQ], F32, kind="ExternalOutput")
    SCR_K = nc.dram_tensor("scr_k", [4, HW], F32)
    SCR_Q = nc.dram_tensor("scr_q", [4, NQ], F32)

    with tile.TileContext(nc) as tc:
        with ExitStack() as ctx:
            pers = ctx.enter_context(tc.tile_pool(name="pers", bufs=1))
            kH = pers.tile([128, HW], BF16, tag="kH")
            kL = pers.tile([128, HW], BF16, tag="kL")
            qH = pers.tile([128, NQ], BF16, tag="qH")
            qL = pers.tile([128, NQ], BF16, tag="qL")
            vtr = pers.tile([128, 128 * 17], F32R, tag="vtr")

            consts = ctx.enter_context(tc.tile_pool(name="consts", bufs=1))
            w17 = consts.tile([7, E], F32)
            nc.sync.dma_start(w17[:], W17.ap())
            w2t = consts.tile([E, E], F32)
            nc.sync.dma_start(w2t[:], W2.ap())
            gcol = consts.tile([E, 1], F32)
            nc.sync.dma_start(gcol[:], LNG.ap())
            bcol = consts.tile([E, 1], F32)
            nc.sync.dma_start(bcol[:], LNB.ap())
            b2col = consts.tile([E, 1], F32)
            nc.sync.dma_start(b2col[:], B2.ap())
            epsc = consts.tile([E, 1], F32)
            nc.vector.memset(epsc[:], LN_EPS)
            ident = consts.tile([128, 128], F32)
            make_identity(nc, ident[:])

            # ---- stage 1 ----
            with ExitStack() as s1:
                P = dict(w17=w17, w2t=w2t, ident=ident, gcol=gcol,
                         bcol=bcol, b2col=b2col, epsc=epsc)
                P["sbBig"] = s1.enter_context(tc.tile_pool(name="sbBig", bufs=1))
                P["sb1"] = s1.enter_context(tc.tile_pool(name="sb1", bufs=3))
                P["hrp"] = s1.enter_context(tc.tile_pool(name="hrp", bufs=10))
                P["psA"] = s1.enter_context(
                    tc.tile_pool(name="psA", bufs=3, space="PSUM"))
                P["psT"] = s1.enter_context(
                    tc.tile_pool(name="psT", bufs=3, space="PSUM"))
                P["psD"] = s1.enter_context(
                    tc.tile_pool(name="psD", bufs=2, space="PSUM"))

                _encode_side(nc, P, HW, KFEAT, SCR_K, kH, kL)
                _encode_side(nc, P, NQ, QFEAT, SCR_Q, qH, qL)

                # V' tile: kv (17,HW) -> vtr[p, c*128+j] (f32r, rounded copy)
                kvf = P["sbBig"].tile([128, 128 * 17], F32, tag="kvf")
                kv_src = KV.ap().rearrange("c (j p) -> p c j", p=128)
                kvf_v = kvf[:].rearrange("p (c j) -> p c j", j=128)
                nc.sync.dma_start(kvf_v, kv_src)
                nc.vector.tensor_copy(vtr[:], kvf[:])
                vtr_cj = vtr[:].rearrange("p (c j) -> p c j", j=128)

            # ---- stage 2 ----
            with ExitStack() as s2:
                ps1 = s2.enter_context(
                    tc.tile_pool(name="ps1", bufs=3, space="PSUM"))
                ps2 = s2.enter_context(
                    tc.tile_pool(name="ps2", bufs=3, space="PSUM"))
                psP = s2.enter_context(
                    tc.tile_pool(name="psP", bufs=2, space="PSUM"))
                mxp = s2.enter_context(tc.tile_pool(name="mxp", bufs=3))
                bcp = s2.enter_context(tc.tile_pool(name="bcp", bufs=2))
                tmp = s2.enter_context(tc.tile_pool(name="tmp", bufs=3))
                exp = s2.enter_context(tc.tile_pool(name="exp", bufs=3))
                outp = s2.enter_context(tc.tile_pool(name="outp", bufs=2))

                NJ = HW // 128
                for qt in range(NQ // 512):
                    qs = slice(qt * 512, (qt + 1) * 512)
                    # pass 1: row max (single bf16 pass)
                    m_prev = None
                    for j in range(NJ):
                        p1 = ps1.tile([128, 512], F32, tag="p1")
                        nc.tensor.matmul(p1[:], kH[:, j * 128:(j + 1) * 128],
                                         qH[:, qs], start=True, stop=True)
                        m_cur = mxp.tile([128, 512], F32, tag="mx")
                        if m_prev is None:
                            nc.vector.tensor_copy(m_cur[:], p1[:])
                        else:
                            nc.vector.tensor_max(m_cur[:], p1[:], m_prev[:])
                        m_prev = m_cur
                    maxbc = bcp.tile([128, 512], F32, tag="maxbc")
                    nc.gpsimd.partition_all_reduce(
                        maxbc[:], m_prev[:], channels=128,
                        reduce_op=bass_isa.ReduceOp.max)

                    # pass 2: exact scores (3 bf16 passes), exp, AV accumulate
                    pP = psP.tile([17, 512], F32, tag="pP")
                    for j in range(NJ):
                        js = slice(j * 128, (j + 1) * 128)
                        p2 = ps2.tile([128, 512], F32, tag="p2")
                        nc.tensor.matmul(p2[:], kH[:, js], qH[:, qs],
                                         start=True, stop=False)
                        nc.tensor.matmul(p2[:], kL[:, js], qH[:, qs],
                                         start=False, stop=False)
                        nc.tensor.matmul(p2[:], kH[:, js], qL[:, qs],
                                         start=False, stop=True)
                        t = tmp.tile([128, 512], F32, tag="t")
                        nc.vector.tensor_sub(t[:], p2[:], maxbc[:])
                        ex = exp.tile([128, 512], F32R, tag="ex")
                        nc.scalar.activation(ex[:], t[:], AF.Exp,
                                             scale=TEMP_INV)
                        nc.tensor.matmul(pP[:], vtr_cj[:, :, j],
                                         ex[:], start=(j == 0),
                                         stop=(j == NJ - 1))
                    pal = outp.tile([17, 512], F32, tag="pal")
                    nc.vector.tensor_copy(pal[:], pP[:])
                    z0 = outp.tile([1, 512], F32, tag="z0")
                    nc.sync.dma_start(z0[:], pal[16:17, :])
                    rz = outp.tile([1, 512], F32, tag="rz")
                    nc.vector.reciprocal(rz[:], z0[:])
                    zb = outp.tile([16, 512], F32, tag="zb")
                    nc.gpsimd.partition_broadcast(zb[:], rz[:], channels=16)
                    onorm = outp.tile([16, 512], F32, tag="onorm")
                    nc.vector.tensor_mul(onorm[:], pal[0:16, :], zb[:])
                    nc.sync.dma_start(ALN.ap()[:, qs], onorm[:])

    nc.compile()
    _cache["nc"] = nc
    return nc


def _host_prep(ego_cls, ego_reg, other_cls, other_reg, w1, b1):
    ego_feat = np.ascontiguousarray(
        np.concatenate([ego_reg[0, 3:7].reshape(4, -1),
                        ego_cls[0].reshape(2, -1)], axis=0)).astype(np.float32)
    oth_feat = np.ascontiguousarray(
        np.concatenate([other_reg[0, 3:7].reshape(4, -1),
                        other_cls[0].reshape(2, -1)], axis=0)).astype(np.float32)
    kv = np.concatenate([other_cls[0].reshape(2, -1),
                         other_reg[0].reshape(14, -1),
                         np.ones((1, HW), np.float32)], axis=0).astype(np.float32)
    w17 = np.concatenate([w1, b1[None, :]], axis=0).astype(np.float32)
    return ego_feat, oth_feat, kv, w17


def kernel(ego_cls, ego_reg, other_cls, other_reg, w1, b1, ln_g, ln_b, w2, b2):
    nc = build()
    ego_feat, oth_feat, kv, w17 = _host_prep(
        ego_cls, ego_reg, other_cls, other_reg, w1, b1)
    lng = np.ascontiguousarray(ln_g.reshape(E, 1)).astype(np.float32)
    lnb = np.ascontiguousarray(ln_b.reshape(E, 1)).astype(np.float32)
    b2c = np.ascontiguousarray(b2.reshape(E, 1)).astype(np.float32)
    w2c = np.ascontiguousarray(w2).astype(np.float32)

    in_maps = []
    for c in range(N_CORES):
        qs = slice(c * NQ, (c + 1) * NQ)
        in_maps.append({
            "qfeat": np.ascontiguousarray(ego_feat[:, qs]),
            "kfeat": oth_feat,
            "kv": kv,
            "w17": w17,
            "w2": w2c,
            "lng": lng,
            "lnb": lnb,
            "b2": b2c,
        })
    res = run_bass_kernel_spmd(nc, in_maps, list(range(N_CORES)))
    aln = np.concatenate([res.results[c]["aln"] for c in range(N_CORES)],
                         axis=1)
    aligned_cls = aln[0:2].reshape(1, 2, 128, 128).astype(np.float32)
    aligned_reg = aln[2:16].reshape(1, 14, 128, 128).astype(np.float32)
    return (aligned_cls, aligned_reg)


# revision 10
# speedup vs baseline: 1.0850x; 1.0850x over previous
"""Trainium2 Bass kernel for nn_DifferentiableFeatureAligner.

Full cross-attention aligner: encode 6-feature descriptors of ego/other
(Linear(6,128)+ReLU+LN+Linear(128,128)), softmax(Q K^T / 0.1) attention
over 16384 keys, weighted sums of other_cls/other_reg channels.

Sharding: sequence-parallel over the 16384 ego queries -> 2048 per core
on 8 NeuronCores; other-side tensors replicated. Host only slices and
concatenates.

Score precision: plain fp32 matmul is 4 cycles/column on the PE. Instead
descriptors are split hi/lo into two bf16 tensors and scores computed as
qh*kh + ql*kh + qh*kl (3 bf16 passes, 1 cycle/column each, ~16-bit
mantissa => ~8e-4 end-to-end error). The softmax row max is found with a
single cheap bf16 pass (softmax is shift-invariant, the max needs no
precision). exp goes through float32r for the 1-cycle AV matmul.
"""
import numpy as np
from contextlib import ExitStack

import concourse.bacc as bacc
import concourse.tile as tile
import concourse.mybir as mybir
import concourse.bass_isa as bass_isa
from concourse.bass_utils import run_bass_kernel_spmd
from concourse.masks import make_identity

F32 = mybir.dt.float32
F32R = mybir.dt.float32r
BF16 = mybir.dt.bfloat16
AF = mybir.ActivationFunctionType
ALU = mybir.AluOpType
PI = float(np.pi)

N_CORES = 8
HW = 16384
NQ = HW // N_CORES      # queries per core
E = 128
TEMP_INV = 10.0
LN_EPS = 1e-5
ONE_ROW = 96            # coordinate mapped to the all-ones direction

def _householder():
    # H = I - 2 v v^T with H @ e_ONE_ROW = 1/sqrt(128): maps the ones
    # direction onto coordinate ONE_ROW. LN output has exactly zero row
    # mean, so row ONE_ROW of H^T @ hnorm^T is ~0 and can carry the
    # ones / -rowmax payload that performs the softmax max subtraction
    # inside the score matmul itself.
    u = np.ones(E, np.float64) / np.sqrt(E)
    v = np.zeros(E, np.float64); v[ONE_ROW] = 1.0
    v = v - u
    v /= np.linalg.norm(v)
    return (np.eye(E) - 2.0 * np.outer(v, v)).astype(np.float32)

H_CONST = _householder()

_cache: dict = {}


def _encode_side(nc, P, n, feat_dram, scratch_dram, descH, descL, side):
    """Encode one side: feat_dram (6,n) -> score-operand [128,n] split
    into bf16 hi (descH) and bf16 lo residual (descL).  n mult of 512.
    side='k': operand = H^T @ hnorm^T (raw LN, no affine; affine/b2/w2
    folded into the q side).  side='q': operand = H^T @ (g * (w2 @
    (w2^T @ (g*hnorm+b)^T + b2)))."""
    nf = n // 128
    sb1 = P["sb1"]

    feat = feat_dram.ap()
    feat_pf = feat.rearrange("c (p f) -> c p f", p=128)

    # ---- feature rows (whole side) -> scratch DRAM [4, n] ----
    th = sb1.tile([128, nf], F32, tag="th")
    nc.sync.dma_start(th[:], feat_pf[3])
    w0 = sb1.tile([128, nf], F32, tag="w0")
    nc.vector.add_range_wrap(w0[:], th[:], 0.0, PI, 2 * PI)
    sn = sb1.tile([128, nf], F32, tag="sn")
    nc.scalar.activation(sn[:], w0[:], AF.Sin)
    w1t = sb1.tile([128, nf], F32, tag="w1t")
    nc.vector.add_range_wrap(w1t[:], th[:], PI / 2, PI, 2 * PI)
    cs = sb1.tile([128, nf], F32, tag="cs")
    nc.scalar.activation(cs[:], w1t[:], AF.Sin)
    c0 = sb1.tile([128, nf], F32, tag="c0")
    nc.sync.dma_start(c0[:], feat_pf[4])
    c1 = sb1.tile([128, nf], F32, tag="c1")
    nc.sync.dma_start(c1[:], feat_pf[5])
    mx = sb1.tile([128, nf], F32, tag="mx")
    nc.vector.tensor_max(mx[:], c0[:], c1[:])
    ones_t = sb1.tile([128, nf], F32, tag="ones_t")
    nc.vector.memset(ones_t[:], 1.0)
    sc = scratch_dram.ap()
    nc.sync.dma_start(sc[0:1, :n], sn[:])
    nc.sync.dma_start(sc[1:2, :n], cs[:])
    nc.sync.dma_start(sc[2:3, :n], mx[:])
    nc.sync.dma_start(sc[3:4, :n], ones_t[:])

    # ---- encode in column ranges of <= 8192 ----
    for col0 in range(0, n, 8192):
        ncols = min(8192, n - col0)
        _encode_range(nc, P, feat, sc, col0, ncols, descH, descL, side)


def _encode_range(nc, P, feat, sc, col0, ncols, descH, descL, side):
    sb1, sbBig, hrp = P["sb1"], P["sbBig"], P["hrp"]
    psA, psT, psD = P["psA"], P["psT"], P["psD"]

    xT = sbBig.tile([7, ncols], F32, tag=f"xT{min(ncols, 8192)}")
    cr = slice(col0, col0 + ncols)
    nc.sync.dma_start(xT[0:3, :], feat[0:3, cr])
    nc.sync.dma_start(xT[3:7, :], sc[:, cr])

    for g in range(ncols // 512):
        st1 = sb1.tile([128, 4], F32, tag="st1")
        st2 = sb1.tile([128, 4], F32, tag="st2")
        hrs = []
        for w in range(4):
            win = g * 4 + w
            ph = psA.tile([128, 128], F32, tag="ph")
            nc.tensor.matmul(ph[:], xT[:, win * 128:(win + 1) * 128],
                             P["w17"][:], start=True, stop=True)
            hr = hrp.tile([128, 128], F32, tag="hr")
            nc.scalar.activation(hr[:], ph[:], AF.Relu,
                                 accum_out=st1[:, w:w + 1])
            sq = sb1.tile([128, 128], F32, tag="sq")
            nc.scalar.activation(sq[:], hr[:], AF.Square,
                                 accum_out=st2[:, w:w + 1])
            hrs.append(hr)
        mu4 = sb1.tile([128, 4], F32, tag="mu4")
        nc.vector.tensor_scalar_mul(mu4[:], st1[:], 1.0 / E)
        ex2 = sb1.tile([128, 4], F32, tag="ex2")
        nc.vector.tensor_scalar_mul(ex2[:], st2[:], 1.0 / E)
        mu2 = sb1.tile([128, 4], F32, tag="mu2")
        nc.vector.tensor_mul(mu2[:], mu4[:], mu4[:])
        var4 = sb1.tile([128, 4], F32, tag="var4")
        nc.vector.tensor_sub(var4[:], ex2[:], mu2[:])
        sd4 = sb1.tile([128, 4], F32, tag="sd4")
        nc.scalar.activation(sd4[:], var4[:], AF.Sqrt, bias=P["epsc"][:])
        rstd4 = sb1.tile([128, 4], F32, tag="rstd4")
        nc.vector.reciprocal(rstd4[:], sd4[:])

        hT = sb1.tile([128, 512], F32, tag="hT")
        for w in range(4):
            hn = sb1.tile([128, 128], F32, tag="hn")
            nc.vector.tensor_scalar(hn[:], hrs[w][:], mu4[:, w:w + 1],
                                    rstd4[:, w:w + 1],
                                    op0=ALU.subtract, op1=ALU.mult)
            pt = psT.tile([128, 128], F32, tag="pt")
            nc.tensor.transpose(pt[:], hn[:], P["ident"][:])
            ws = slice(w * 128, (w + 1) * 128)
            if side == "q":
                nc.vector.tensor_scalar(hT[:, ws], pt[:],
                                        P["gcol"][:], P["bcol"][:],
                                        op0=ALU.mult, op1=ALU.add)
            else:
                nc.vector.tensor_copy(hT[:, ws], pt[:])
        gs = slice(col0 + g * 512, col0 + (g + 1) * 512)
        if side == "q":
            pd = psD.tile([128, 512], F32, tag="pd")
            nc.tensor.matmul(pd[:], P["w2t"][:], hT[:], start=True, stop=True)
            d1 = sb1.tile([128, 512], F32, tag="d1")
            nc.scalar.activation(d1[:], pd[:], AF.Identity, bias=P["b2col"][:])
            pd2 = psD.tile([128, 512], F32, tag="pd2")
            nc.tensor.matmul(pd2[:], P["w2T"][:], d1[:], start=True, stop=True)
            t3 = sb1.tile([128, 512], F32, tag="t3")
            nc.vector.tensor_scalar_mul(t3[:], pd2[:], P["gcol"][:])
            pd3 = psD.tile([128, 512], F32, tag="pd3")
            nc.tensor.matmul(pd3[:], P["hc"][:], t3[:], start=True, stop=True)
            dc = sb1.tile([128, 512], F32, tag="dc")
            nc.scalar.activation(dc[:], pd3[:], AF.Copy)
        else:
            pd = psD.tile([128, 512], F32, tag="pd")
            nc.tensor.matmul(pd[:], P["hc"][:], hT[:], start=True, stop=True)
            dc = sb1.tile([128, 512], F32, tag="dc")
            nc.scalar.activation(dc[:], pd[:], AF.Copy)
        nc.vector.tensor_copy(descH[:, gs], dc[:])
        nc.vector.tensor_sub(descL[:, gs], dc[:], descH[:, gs])


def build():
    if "nc" in _cache:
        return _cache["nc"]
    nc = bacc.Bacc("TRN2", target_bir_lowering=False, debug=False)

    QFEAT = nc.dram_tensor("qfeat", [6, NQ], F32, kind="ExternalInput")
    KFEAT = nc.dram_tensor("kfeat", [6, HW], F32, kind="ExternalInput")
    KV = nc.dram_tensor("kv", [17, HW], F32, kind="ExternalInput")
    W17 = nc.dram_tensor("w17", [7, E], F32, kind="ExternalInput")
    W2 = nc.dram_tensor("w2", [E, E], F32, kind="ExternalInput")
    W2T = nc.dram_tensor("w2T", [E, E], F32, kind="ExternalInput")
    HC = nc.dram_tensor("hc", [E, E], F32, kind="ExternalInput")
    LNG = nc.dram_tensor("lng", [E, 1], F32, kind="ExternalInput")
    LNB = nc.dram_tensor("lnb", [E, 1], F32, kind="ExternalInput")
    B2 = nc.dram_tensor("b2", [E, 1], F32, kind="ExternalInput")
    ALN = nc.dram_tensor("aln", [16, NQ], F32, kind="ExternalOutput")
    SCR_K = nc.dram_tensor("scr_k", [4, HW], F32)
    SCR_Q = nc.dram_tensor("scr_q", [4, NQ], F32)

    with tile.TileContext(nc) as tc:
        with ExitStack() as ctx:
            pers = ctx.enter_context(tc.tile_pool(name="pers", bufs=1))
            kH = pers.tile([128, HW], BF16, tag="kH")
            kL = pers.tile([128, HW], BF16, tag="kL")
            qH = pers.tile([128, NQ], BF16, tag="qH")
            qL = pers.tile([128, NQ], BF16, tag="qL")
            vtr = pers.tile([128, 128 * 17], F32R, tag="vtr")

            consts = ctx.enter_context(tc.tile_pool(name="consts", bufs=1))
            w17 = consts.tile([7, E], F32)
            nc.sync.dma_start(w17[:], W17.ap())
            w2t = consts.tile([E, E], F32)
            nc.sync.dma_start(w2t[:], W2.ap())
            w2Tt = consts.tile([E, E], F32)
            nc.sync.dma_start(w2Tt[:], W2T.ap())
            hct = consts.tile([E, E], F32)
            nc.sync.dma_start(hct[:], HC.ap())
            gcol = consts.tile([E, 1], F32)
            nc.sync.dma_start(gcol[:], LNG.ap())
            bcol = consts.tile([E, 1], F32)
            nc.sync.dma_start(bcol[:], LNB.ap())
            b2col = consts.tile([E, 1], F32)
            nc.sync.dma_start(b2col[:], B2.ap())
            epsc = consts.tile([E, 1], F32)
            nc.vector.memset(epsc[:], LN_EPS)
            ident = consts.tile([128, 128], F32)
            make_identity(nc, ident[:])

            # ---- stage 1 ----
            with ExitStack() as s1:
                P = dict(w17=w17, w2t=w2t, w2T=w2Tt, hc=hct, ident=ident,
                         gcol=gcol, bcol=bcol, b2col=b2col, epsc=epsc)
                P["sbBig"] = s1.enter_context(tc.tile_pool(name="sbBig", bufs=1))
                P["sb1"] = s1.enter_context(tc.tile_pool(name="sb1", bufs=3))
                P["hrp"] = s1.enter_context(tc.tile_pool(name="hrp", bufs=10))
                P["psA"] = s1.enter_context(
                    tc.tile_pool(name="psA", bufs=2, space="PSUM"))
                P["psT"] = s1.enter_context(
                    tc.tile_pool(name="psT", bufs=2, space="PSUM"))
                P["psD"] = s1.enter_context(
                    tc.tile_pool(name="psD", bufs=1, space="PSUM"))

                _encode_side(nc, P, HW, KFEAT, SCR_K, kH, kL, "k")
                _encode_side(nc, P, NQ, QFEAT, SCR_Q, qH, qL, "q")
                # payload rows: ones on k-hi, zero on k-lo / q-hi; q-lo
                # row gets -rowmax per query tile in stage 2.
                nc.vector.memset(kH[ONE_ROW:ONE_ROW + 1, :], 1.0)
                nc.vector.memset(kL[ONE_ROW:ONE_ROW + 1, :], 0.0)
                nc.vector.memset(qH[ONE_ROW:ONE_ROW + 1, :], 0.0)

                # V' tile: kv (17,HW) -> vtr[p, c*128+j] (f32r, rounded copy)
                vtr_cj = vtr[:].rearrange("p (c j) -> p c j", j=128)
                kvf = P["sbBig"].tile([128, 128 * 17], F32, tag="kvf")
                kv_src = KV.ap().rearrange("c (j p) -> p c j", p=128)
                kvf_v = kvf[:].rearrange("p (c j) -> p c j", j=128)
                nc.sync.dma_start(kvf_v, kv_src)
                nc.vector.tensor_copy(vtr[:], kvf[:])

            # ---- stage 2 ----
            with ExitStack() as s2:
                ps1 = s2.enter_context(
                    tc.tile_pool(name="ps1", bufs=2, space="PSUM"))
                ps2 = s2.enter_context(
                    tc.tile_pool(name="ps2", bufs=3, space="PSUM"))
                psP = s2.enter_context(
                    tc.tile_pool(name="psP", bufs=1, space="PSUM"))
                mxp = s2.enter_context(tc.tile_pool(name="mxp", bufs=3))
                bcp = s2.enter_context(tc.tile_pool(name="bcp", bufs=2))
                exp = s2.enter_context(tc.tile_pool(name="exp", bufs=3))
                outp = s2.enter_context(tc.tile_pool(name="outp", bufs=2))

                NJ = HW // 128
                for qt in range(NQ // 512):
                    qs = slice(qt * 512, (qt + 1) * 512)
                    # pass 1: row max (single bf16 pass, 2 chunks per bank
                    # pair so the DVE running max runs on [128,1024])
                    m_prev = None
                    for jj in range(NJ // 2):
                        p1 = ps1.tile([128, 1024], F32, tag="p1")
                        nc.tensor.matmul(p1[:, 0:512],
                                         kH[:, (2 * jj) * 128:(2 * jj + 1) * 128],
                                         qH[:, qs], start=True, stop=True)
                        nc.tensor.matmul(p1[:, 512:1024],
                                         kH[:, (2 * jj + 1) * 128:(2 * jj + 2) * 128],
                                         qH[:, qs], start=True, stop=True)
                        m_cur = mxp.tile([128, 1024], F32, tag="mx")
                        if m_prev is None:
                            nc.vector.tensor_copy(m_cur[:], p1[:])
                        else:
                            nc.vector.tensor_max(m_cur[:], p1[:], m_prev[:])
                        m_prev = m_cur
                    mfin = bcp.tile([128, 512], F32, tag="mfin")
                    nc.vector.tensor_max(mfin[:], m_prev[:, 0:512],
                                         m_prev[:, 512:1024])
                    maxbc = bcp.tile([128, 512], F32, tag="maxbc")
                    nc.gpsimd.partition_all_reduce(
                        maxbc[:], mfin[:], channels=128,
                        reduce_op=bass_isa.ReduceOp.max)
                    # -rowmax payload into the q-lo ones coordinate: the
                    # third score matmul then subtracts m_q in PSUM.
                    nc.vector.tensor_scalar_mul(qL[ONE_ROW:ONE_ROW + 1, qs],
                                                maxbc[0:1, :], -1.0)

                    # pass 2: exact scores (3 bf16 passes incl. -max), exp
                    # straight from PSUM, AV accumulate
                    pP = psP.tile([17, 512], F32, tag="pP")
                    for j in range(NJ):
                        js = slice(j * 128, (j + 1) * 128)
                        p2 = ps2.tile([128, 512], F32, tag="p2")
                        nc.tensor.matmul(p2[:], kH[:, js], qH[:, qs],
                                         start=True, stop=False)
                        nc.tensor.matmul(p2[:], kL[:, js], qH[:, qs],
                                         start=False, stop=False)
                        nc.tensor.matmul(p2[:], kH[:, js], qL[:, qs],
                                         start=False, stop=True)
                        ex = exp.tile([128, 512], F32R, tag="ex")
                        nc.scalar.activation(ex[:], p2[:], AF.Exp,
                                             scale=TEMP_INV)
                        nc.tensor.matmul(pP[:], vtr_cj[:, :, j],
                                         ex[:], start=(j == 0),
                                         stop=(j == NJ - 1))
                    pal = outp.tile([17, 512], F32, tag="pal")
                    nc.vector.tensor_copy(pal[:], pP[:])
                    z0 = outp.tile([1, 512], F32, tag="z0")
                    nc.sync.dma_start(z0[:], pal[16:17, :])
                    rz = outp.tile([1, 512], F32, tag="rz")
                    nc.vector.reciprocal(rz[:], z0[:])
                    zb = outp.tile([16, 512], F32, tag="zb")
                    nc.gpsimd.partition_broadcast(zb[:], rz[:], channels=16)
                    onorm = outp.tile([16, 512], F32, tag="onorm")
                    nc.vector.tensor_mul(onorm[:], pal[0:16, :], zb[:])
                    nc.sync.dma_start(ALN.ap()[:, qs], onorm[:])

    nc.compile()
    _cache["nc"] = nc
    return nc


def _host_prep(ego_cls, ego_reg, other_cls, other_reg, w1, b1):
    ego_feat = np.ascontiguousarray(
        np.concatenate([ego_reg[0, 3:7].reshape(4, -1),
                        ego_cls[0].reshape(2, -1)], axis=0)).astype(np.float32)
    oth_feat = np.ascontiguousarray(
        np.concatenate([other_reg[0, 3:7].reshape(4, -1),
                        other_cls[0].reshape(2, -1)], axis=0)).astype(np.float32)
    kv = np.concatenate([other_cls[0].reshape(2, -1),
                         other_reg[0].reshape(14, -1),
                         np.ones((1, HW), np.float32)], axis=0).astype(np.float32)
    w17 = np.concatenate([w1, b1[None, :]], axis=0).astype(np.float32)
    return ego_feat, oth_feat, kv, w17


def kernel(ego_cls, ego_reg, other_cls, other_reg, w1, b1, ln_g, ln_b, w2, b2):
    nc = build()
    ego_feat, oth_feat, kv, w17 = _host_prep(
        ego_cls, ego_reg, other_cls, other_reg, w1, b1)
    lng = np.ascontiguousarray(ln_g.reshape(E, 1)).astype(np.float32)
    lnb = np.ascontiguousarray(ln_b.reshape(E, 1)).astype(np.float32)
    b2c = np.ascontiguousarray(b2.reshape(E, 1)).astype(np.float32)
    w2c = np.ascontiguousarray(w2).astype(np.float32)
    w2Tc = np.ascontiguousarray(w2c.T)

    in_maps = []
    for c in range(N_CORES):
        qs = slice(c * NQ, (c + 1) * NQ)
        in_maps.append({
            "qfeat": np.ascontiguousarray(ego_feat[:, qs]),
            "kfeat": oth_feat,
            "kv": kv,
            "w17": w17,
            "w2": w2c,
            "w2T": w2Tc,
            "hc": H_CONST,
            "lng": lng,
            "lnb": lnb,
            "b2": b2c,
        })
    res = run_bass_kernel_spmd(nc, in_maps, list(range(N_CORES)))
    aln = np.concatenate([res.results[c]["aln"] for c in range(N_CORES)],
                         axis=1)
    aligned_cls = aln[0:2].reshape(1, 2, 128, 128).astype(np.float32)
    aligned_reg = aln[2:16].reshape(1, 14, 128, 128).astype(np.float32)
    return (aligned_cls, aligned_reg)
